# revision 35
# baseline (speedup 1.0000x reference)
"""Trainium2 Bass kernel for nn_Attention_21088289423660 (sparse_attention).

Reference computation (per token t = (b, n, m), feature dim D=256):
    kh = Wk^T k_t ; qh = Wq^T q_t ; v = Wv^T kh
    S  = kh - qh + pos_t
    attn = sigmoid(W2^T relu(W1^T S + b1) + b2)      (mask is all-ones)
    out  = Wo^T ((v + pos_t) * attn) + bo

Folded algebra (S is never materialized):
    h1  = A^T k + Bn^T q + W1^T pos + b1    A = Wk@W1, Bn = -Wq@W1
    v   = Wkv^T k                            Wkv = Wk@Wv
    h2  = W2^T relu(h1)
    attn = sigmoid(h2 + b2)
    out  = Wo^T ((v + pos) * attn)
The q term runs as one fp8 e4m3 DoubleRow matmul (q8 + Bn8 replace the
bf16 pair; q is h1-only so the bf16 q copy is dropped from DMA
entirely); everything else is bf16. 13 bf16 MMs + 1 DR MM per
512-token tile vs 20 unfused (rel_l2 ~9e-3 vs the 2e-2 budget).

Sharding: data-parallel over 8 cores; core c handles batch b=c//2 and
N-half (c%2) -> 16384 tokens/core, weights replicated.

Compute dtype: bf16 (PSUM accumulation fp32), device output bf16,
host adds bo and widens to fp32.
"""

import os
import sys

for _p in (
    "/root/.axon_site",
    "/root/.axon_site/_ro/trn_rl_repo",
    "/root/.axon_site/_ro/pypackages",
    "/opt/trn_rl_repo",
):
    if os.path.isdir(_p) and _p not in sys.path:
        sys.path.append(_p)

import numpy as np
import ml_dtypes
from contextlib import ExitStack

import concourse.bass as bass
import concourse.tile as tile
import concourse.bacc as bacc
from concourse import mybir
from concourse import bass_utils

BF16 = ml_dtypes.bfloat16

B, DIM, N, M = 4, 256, 4096, 8
NCORES = 8
NT = (B * N * M) // NCORES          # tokens per core = 16384
P = 128                              # partitions
NDC = DIM // P                       # d-chunks = 2
CHUNK = 2048                         # tokens per DMA chunk
FD = 512                             # tokens per matmul tile
F32 = mybir.dt.float32
BF = mybir.dt.bfloat16
F8 = mybir.dt.float8e4
FP8 = ml_dtypes.float8_e4m3

WARMUP_MMS = int(os.environ.get("KERNEL_WARMUP_MMS", "8"))

_CACHED_NC = None


def _build_nc():
    """Build and compile the per-core Bass program (SPMD, identical on all cores).

    Software-pipelined at depth 4; a dummy warm-up matmul accumulation group
    runs during the DMA head so the PE HAM clock gate is at 8/8 before real
    matmuls start. Weights are packed into few DRAM tensors because each
    DMA issue occupies its queue ~0.7us.
    """
    nc = bacc.Bacc("TRN2", target_bir_lowering=False, debug=False)

    q8_d = nc.dram_tensor("q8s", (NDC, P, NT), F8, kind="ExternalInput").ap()
    k_d = nc.dram_tensor("ks", (NDC, P, NT), BF, kind="ExternalInput").ap()
    pos_d = nc.dram_tensor("poss", (NDC, P, NT), BF, kind="ExternalInput").ap()
    # wh1 = [A | W1] bf16 stationaries; Bn is fp8 (DoubleRow with q8)
    wh1_d = nc.dram_tensor("wh1", (NDC, P, 2 * P), BF, kind="ExternalInput").ap()
    wbn8_d = nc.dram_tensor("wbn8", (P, NDC, P), F8, kind="ExternalInput").ap()
    wvo_d = nc.dram_tensor("wvo", (NDC, P, 2 * DIM), BF, kind="ExternalInput").ap()
    w2_d = nc.dram_tensor("w2", (P, DIM), BF, kind="ExternalInput").ap()
    bias_d = nc.dram_tensor("bias", (P, 3), F32, kind="ExternalInput").ap()
    out_d = nc.dram_tensor("out", (NDC, P, NT), BF, kind="ExternalOutput").ap()

    k_r = k_d.rearrange("c p t -> p c t")
    q8_r = q8_d.rearrange("c p t -> p c t")
    pos_r = pos_d.rearrange("c p t -> p c t")
    out_r = out_d.rearrange("c p t -> p c t")

    AF = mybir.ActivationFunctionType
    n_chunks = NT // CHUNK              # 8
    ipc = CHUNK // FD                   # iters per chunk = 4
    n_total = NT // FD                  # global iterations = 32

    with tile.TileContext(nc) as tc, ExitStack() as ctx:
        wpool = ctx.enter_context(tc.tile_pool(name="wpool", bufs=1))
        iopool = ctx.enter_context(tc.tile_pool(name="iopool", bufs=2))
        mid = ctx.enter_context(tc.tile_pool(name="mid", bufs=3))
        pp = ctx.enter_context(tc.tile_pool(name="pp", bufs=1, space="PSUM"))

        # --- warm-up scratch (no DMA deps; PE can start immediately) ---
        wu_w = wpool.tile([P, P], BF, tag="wu_w", name="wu_w")
        nc.gpsimd.memset(wu_w[:], 0.0)
        wu_in = wpool.tile([P, FD], BF, tag="wu_in", name="wu_in")
        nc.gpsimd.memset(wu_in[:], 0.0)

        # --- ACT spline tables FIRST on the scalar queue: relu(0) gates the
        # h1 PSUM rotation, so nothing may sit ahead of it on ACT ---
        dum_out = wpool.tile([P, 1], BF, tag="dum_out", name="dum_out")
        nc.scalar.activation(dum_out[:], wu_w[:, 0:1], AF.Relu)
        nc.scalar.activation(dum_out[:], wu_w[:, 0:1], AF.Sigmoid)

        # --- weights / biases resident in SBUF (packed, one DMA each);
        # h1-path weights ride the sync ring ahead of the k slices ---
        wh1_t = wpool.tile([P, NDC, 2 * P], BF, tag="wh1", name="wh1")
        nc.sync.dma_start(wh1_t[:], wh1_d.rearrange("c p e -> p c e"))
        wbn8_t = wpool.tile([P, NDC, P], F8, tag="wbn8", name="wbn8")
        nc.sync.dma_start(wbn8_t[:], wbn8_d[:])
        w2_t = wpool.tile([P, DIM], BF, tag="w2", name="w2")
        nc.scalar.dma_start(w2_t[:], w2_d[:])
        bias_t = wpool.tile([P, 3], F32, tag="bias", name="bias")
        nc.scalar.dma_start(bias_t[:], bias_d[:])
        wvo_t = wpool.tile([P, NDC, 2 * DIM], BF, tag="wvo", name="wvo")
        nc.gpsimd.dma_start(wvo_t[:], wvo_d.rearrange("c p e -> p c e"))

        def a_w(c):
            return wh1_t[:, c, 0:P]

        def w1_w(c):
            return wh1_t[:, c, P:2 * P]

        def wkv_w(c, e):
            return wvo_t[:, c, e * P:(e + 1) * P]

        def wo_w(c, e):
            return wvo_t[:, c, DIM + e * P:DIM + (e + 1) * P]

        # --- HAM warm-up: one accumulation group of dummy matmuls (no
        # per-MM semaphores, back-to-back on the PE) ---
        wu_ps = pp.tile([P, NDC, FD], F32, tag="xo", bufs=1, name="wu_ps")
        for i in range(WARMUP_MMS):
            nc.tensor.matmul(wu_ps[:, 0, :], wu_w[:], wu_in[:],
                             start=(i == 0), stop=(i == WARMUP_MMS - 1))

        io = {}        # chunk -> dict of io tiles
        st = {}        # global iter -> dict of stage tiles

        def load_chunk(ci):
            csl = bass.ts(ci, CHUNK)
            kt = iopool.tile([P, NDC, CHUNK], BF, tag="kt", bufs=3, name="kt")
            qt = iopool.tile([P, NDC, CHUNK], F8, tag="qt", bufs=3, name="qt")
            post = iopool.tile([P, NDC, CHUNK], BF, tag="post", bufs=3, name="post")
            if ci == 0:
                # first 512-token slice lands fastest so s1(0) starts early;
                # scalar/ACT queue is idle at the head, so it carries pos;
                # steady state keeps it DMA-free
                for sl in (slice(0, FD), slice(FD, CHUNK)):
                    nc.sync.dma_start(kt[:, :, sl], k_r[:, :, sl])
                    nc.scalar.dma_start(post[:, :, sl], pos_r[:, :, sl])
                    nc.gpsimd.dma_start(qt[:, :, sl], q8_r[:, :, sl])
            else:
                nc.sync.dma_start(kt[:], k_r[:, :, csl])
                nc.sync.dma_start(post[:], pos_r[:, :, csl])
                nc.gpsimd.dma_start(qt[:], q8_r[:, :, csl])
            outt = iopool.tile([P, NDC, CHUNK], BF, tag="outt", bufs=2, name="outt")
            io[ci] = {"kt": kt, "qt": qt, "post": post, "outt": outt}

        def s1(j):
            ci, it = divmod(j, ipc)
            t = io[ci]
            tsl = bass.ts(it, FD)
            h1w = pp.tile([P, FD], F32, tag="h1", bufs=2, name="h1w")
            nc.tensor.matmul(h1w[:], a_w(0), t["kt"][:, 0, tsl],
                             start=True, stop=False)
            nc.tensor.matmul(h1w[:], a_w(1), t["kt"][:, 1, tsl],
                             start=False, stop=False)
            # q term: one fp8 DoubleRow matmul contracts both 128-chunks
            nc.tensor.matmul(h1w[:], wbn8_t[:], t["qt"][:, :, tsl],
                             start=False, stop=False,
                             perf_mode=mybir.MatmulPerfMode.DoubleRow)
            nc.tensor.matmul(h1w[:], w1_w(0), t["post"][:, 0, tsl],
                             start=False, stop=False)
            nc.tensor.matmul(h1w[:], w1_w(1), t["post"][:, 1, tsl],
                             start=False, stop=True)
            vw = pp.tile([P, NDC, FD], F32, tag="v", bufs=1, name="vw")
            for e in range(NDC):
                nc.tensor.matmul(vw[:, e, :], wkv_w(0, e), t["kt"][:, 0, tsl],
                                 start=True, stop=False)
                nc.tensor.matmul(vw[:, e, :], wkv_w(1, e), t["kt"][:, 1, tsl],
                                 start=False, stop=True)
            vp_t = mid.tile([P, NDC, FD], BF, tag="vp", bufs=4, name="vp_t")
            nc.vector.tensor_add(vp_t[:], vw[:], t["post"][:, :, tsl])
            st[j] = {"h1": h1w, "vp": vp_t}

        def s2(j):
            # relu -> h2 matmuls -> sigmoid (merged MLP stage)
            s = st[j]
            h1r = mid.tile([P, FD], BF, tag="h1r", bufs=2, name="h1r")
            nc.scalar.activation(h1r[:], s["h1"][:], AF.Relu, bias=bias_t[:, 0:1])
            h2w = pp.tile([P, NDC, FD], F32, tag="h2", bufs=1, name="h2w")
            for e in range(NDC):
                esl = bass.ts(e, P)
                nc.tensor.matmul(h2w[:, e, :], w2_t[:, esl], h1r[:],
                                 start=True, stop=True)
            at_t = mid.tile([P, NDC, FD], BF, tag="at", bufs=2, name="at_t")
            for e in range(NDC):
                nc.scalar.activation(at_t[:, e, :], h2w[:, e, :], AF.Sigmoid,
                                     bias=bias_t[:, 1 + e:2 + e])
            s["at"] = at_t

        def s4(j):
            s = st[j]
            g_t = mid.tile([P, NDC, FD], BF, tag="g", bufs=2, name="g_t")
            nc.gpsimd.tensor_mul(g_t[:, 0, :], s["vp"][:, 0, :], s["at"][:, 0, :])
            nc.vector.tensor_mul(g_t[:, 1, :], s["vp"][:, 1, :], s["at"][:, 1, :])
            s["g"] = g_t

        def s5(j):
            ci, it = divmod(j, ipc)
            t = io[ci]
            tsl = bass.ts(it, FD)
            s = st[j]
            xo = pp.tile([P, NDC, FD], F32, tag="xo", bufs=1, name="xo")
            for e in range(NDC):
                nc.tensor.matmul(xo[:, e, :], wo_w(0, e), s["g"][:, 0, :],
                                 start=True, stop=False)
                nc.tensor.matmul(xo[:, e, :], wo_w(1, e), s["g"][:, 1, :],
                                 start=False, stop=True)
            nc.vector.tensor_copy(t["outt"][:, 0, tsl], xo[:, 0, :])
            nc.scalar.copy(t["outt"][:, 1, tsl], xo[:, 1, :])
            del st[j]
            # out DMA: per 1024 tokens; last chunk per 512-half so the final
            # transfer starts right after its own copy, shrinking the tail
            base = ci * CHUNK
            if ci == n_chunks - 1:
                gsl = slice(base + it * FD, base + (it + 1) * FD)
                nc.sync.dma_start(out_r[:, 0, gsl], t["outt"][:, 0, tsl])
                nc.sync.dma_start(out_r[:, 1, gsl], t["outt"][:, 1, tsl])
            elif it % 2 == 1:
                sl2 = slice((it - 1) * FD, (it + 1) * FD)
                nc.sync.dma_start(out_r[:, :, base + (it - 1) * FD:base + (it + 1) * FD],
                                  t["outt"][:, :, sl2])

        for t in range(n_total + 2):
            if t - 2 >= 0 and t - 2 < n_total:
                s4(t - 2)
            if t < n_total:
                if t % ipc == 0:
                    ci = t // ipc
                    if ci == 0:
                        load_chunk(0)
                        load_chunk(1)
                    elif ci + 1 < n_chunks:
                        load_chunk(ci + 1)
                s1(t)
            if t - 1 >= 0 and t - 1 < n_total:
                s2(t - 1)
            if t - 2 >= 0 and t - 2 < n_total:
                s5(t - 2)

    nc.compile()
    return nc


def _get_nc():
    global _CACHED_NC
    if _CACHED_NC is None:
        _CACHED_NC = _build_nc()
    return _CACHED_NC


def _prep_in_maps(q, k, pos, Wq, Wk, Wv, W1, b1, W2, b2, Wo, bo):
    q = np.asarray(q, dtype=np.float32)
    k = np.asarray(k, dtype=np.float32)
    pos = np.asarray(pos, dtype=np.float32)
    Wq32 = np.asarray(Wq, np.float32)
    Wk32 = np.asarray(Wk, np.float32)
    W132 = np.asarray(W1, np.float32)

    A = (Wk32 @ W132).astype(BF16)                    # (256, 128)
    wh1 = np.concatenate([A, W132.astype(BF16)], axis=1)         # (256, 256)
    Bn = -(Wq32 @ W132)                               # (256, 128) fp32
    wbn8 = np.ascontiguousarray(
        Bn.reshape(NDC, P, P).transpose(1, 0, 2).astype(FP8))    # (128, 2, 128)
    Wkv = (Wk32 @ np.asarray(Wv, np.float32)).astype(BF16)   # (256, 256)
    Wob = np.asarray(Wo, np.float32).astype(BF16)
    wvo = np.concatenate([Wkv, Wob], axis=1)          # (256, 512)
    bias = np.stack([np.asarray(b1, np.float32),
                     np.asarray(b2, np.float32)[:P],
                     np.asarray(b2, np.float32)[P:]], axis=1)  # (128, 3)

    weights = {
        "wh1": np.ascontiguousarray(wh1).reshape(NDC, P, 2 * P),
        "wbn8": wbn8,
        "wvo": np.ascontiguousarray(wvo).reshape(NDC, P, 2 * DIM),
        "w2": np.ascontiguousarray(np.asarray(W2, np.float32).astype(BF16)).reshape(P, DIM),
        "bias": np.ascontiguousarray(bias.astype(np.float32)),
    }

    nhalf = N // 2
    in_maps = []
    for c in range(NCORES):
        b = c // 2
        n0 = (c % 2) * nhalf
        qs = q[b, :, n0:n0 + nhalf, :].reshape(DIM, NT)
        ks = k[b, :, n0:n0 + nhalf, :].reshape(DIM, NT)
        ps = np.ascontiguousarray(
            pos[b, n0:n0 + nhalf].reshape(NT, DIM).T
        )
        m = dict(weights)
        m["q8s"] = qs.astype(FP8).reshape(NDC, P, NT)
        m["ks"] = ks.astype(BF16).reshape(NDC, P, NT)
        m["poss"] = ps.astype(BF16).reshape(NDC, P, NT)
        in_maps.append(m)
    return in_maps


def _run(in_maps, trace=False, **kwargs):
    nc = _get_nc()
    return bass_utils.run_bass_kernel_spmd(
        nc, in_maps, core_ids=list(range(NCORES)), trace=trace, **kwargs
    )


def _assemble(results, bo, mask):
    bo = np.asarray(bo, np.float32)
    out = np.empty((B, DIM, N, M), dtype=np.float32)
    nhalf = N // 2
    for c in range(NCORES):
        b = c // 2
        n0 = (c % 2) * nhalf
        r = results[c]["out"].reshape(DIM, nhalf, M).astype(np.float32)
        r += bo[:, None, None]
        out[b, :, n0:n0 + nhalf, :] = r
    mask = np.asarray(mask)
    if not np.all(mask != 0):
        # masked positions: sigmoid(-1e9)=0 -> x=0 -> out = bo
        zb, zn, zm = np.nonzero(mask[..., 0] == 0)
        out[zb, :, zn, zm] = bo[None, :]
    return out


def kernel(q, k, pos, mask, Wq, Wk, Wv, W1, b1, W2, b2, Wo, bo):
    in_maps = _prep_in_maps(q, k, pos, Wq, Wk, Wv, W1, b1, W2, b2, Wo, bo)
    res = _run(in_maps)
    return _assemble(res.results, bo, mask)


def install_profile_hook():
    """Register the axon NTFF profiling hook (antenv.axon_hooks shim) so
    run_bass_kernel_spmd(trace=True) yields exec_time_ns + perfetto trace."""
    import types

    try:
        import antenv.axon_hooks  # noqa: F401
        return True
    except ImportError:
        pass
    try:
        from trn_agent_boot.trn_boot import _ntff_profile_via_ctypes
    except ImportError:
        return False
    hook = _ntff_profile_via_ctypes("/opt/axon/libaxon_pjrt.so")
    if hook is None:
        return False
    mod = types.ModuleType("antenv.axon_hooks")
    mod.get_axon_ntff_profile_hook = lambda: hook
    mod.set_axon_ntff_profile_hook = lambda h: None
    import antenv

    sys.modules["antenv.axon_hooks"] = mod
    antenv.axon_hooks = mod
    # artifact upload has no share reachable from this container
    bass_utils.upload_artifacts = lambda tmpdir: tmpdir
    return True


# revision 38
# speedup vs baseline: 1.1400x; 1.1400x over previous
"""Trainium2 Bass kernel for nn_Attention_21088289423660 (sparse_attention).

Reference computation (per token t = (b, n, m), feature dim D=256):
    kh = Wk^T k_t ; qh = Wq^T q_t ; v = Wv^T kh
    S  = kh - qh + pos_t
    attn = sigmoid(W2^T relu(W1^T S + b1) + b2)      (mask is all-ones)
    out  = Wo^T ((v + pos_t) * attn) + bo

Folded algebra (S is never materialized):
    h1  = A^T k + Bn^T q + W1^T pos + b1    A = Wk@W1, Bn = -Wq@W1
    v   = Wkv^T k                            Wkv = Wk@Wv
    h2  = W2^T relu(h1)
    attn = sigmoid(h2 + b2)
    out  = Wo^T ((v + pos) * attn)
The q term runs as one fp8 e4m3 DoubleRow matmul (q8 + Bn8 replace the
bf16 pair; q is h1-only so the bf16 q copy is dropped from DMA
entirely); everything else is bf16. 13 bf16 MMs + 1 DR MM per
512-token tile vs 20 unfused (rel_l2 ~9e-3 vs the 2e-2 budget).

Sharding: data-parallel over 8 cores; core c handles batch b=c//2 and
N-half (c%2) -> 16384 tokens/core, weights replicated.

Compute dtype: bf16 (PSUM accumulation fp32), device output bf16,
host adds bo and widens to fp32.
"""

import os
import sys

for _p in (
    "/root/.axon_site",
    "/root/.axon_site/_ro/trn_rl_repo",
    "/root/.axon_site/_ro/pypackages",
    "/opt/trn_rl_repo",
):
    if os.path.isdir(_p) and _p not in sys.path:
        sys.path.append(_p)

import numpy as np
import ml_dtypes
from contextlib import ExitStack

import concourse.bass as bass
import concourse.tile as tile
import concourse.bacc as bacc
from concourse import mybir
from concourse import bass_utils

BF16 = ml_dtypes.bfloat16

B, DIM, N, M = 4, 256, 4096, 8
NCORES = 8
NT = (B * N * M) // NCORES          # tokens per core = 16384
P = 128                              # partitions
NDC = DIM // P                       # d-chunks = 2
CHUNK = 2048                         # tokens per DMA chunk
FD = 512                             # tokens per matmul tile
F32 = mybir.dt.float32
BF = mybir.dt.bfloat16
F8 = mybir.dt.float8e4
FP8 = ml_dtypes.float8_e4m3

WARMUP_MMS = int(os.environ.get("KERNEL_WARMUP_MMS", "7"))

_CACHED_NC = None


def _build_nc():
    """Build and compile the per-core Bass program (SPMD, identical on all cores).

    Software-pipelined at depth 4; a dummy warm-up matmul accumulation group
    runs during the DMA head so the PE HAM clock gate is at 8/8 before real
    matmuls start. Weights are packed into few DRAM tensors because each
    DMA issue occupies its queue ~0.7us.
    """
    nc = bacc.Bacc("TRN2", target_bir_lowering=False, debug=False)

    q8_d = nc.dram_tensor("q8s", (NDC, P, NT), F8, kind="ExternalInput").ap()
    k_d = nc.dram_tensor("ks", (NDC, P, NT), BF, kind="ExternalInput").ap()
    pos_d = nc.dram_tensor("poss", (NDC, P, NT), BF, kind="ExternalInput").ap()
    # wh1 = [A | W1] bf16 stationaries; Bn is fp8 (DoubleRow with q8)
    wh1_d = nc.dram_tensor("wh1", (NDC, P, 2 * P), BF, kind="ExternalInput").ap()
    wbn8_d = nc.dram_tensor("wbn8", (P, NDC, P), F8, kind="ExternalInput").ap()
    wvo_d = nc.dram_tensor("wvo", (NDC, P, 2 * DIM), BF, kind="ExternalInput").ap()
    w2_d = nc.dram_tensor("w2", (P, DIM), BF, kind="ExternalInput").ap()
    bias_d = nc.dram_tensor("bias", (P, 3), F32, kind="ExternalInput").ap()
    out_d = nc.dram_tensor("out", (NDC, P, NT), BF, kind="ExternalOutput").ap()

    k_r = k_d.rearrange("c p t -> p c t")
    q8_r = q8_d.rearrange("c p t -> p c t")
    pos_r = pos_d.rearrange("c p t -> p c t")
    out_r = out_d.rearrange("c p t -> p c t")

    AF = mybir.ActivationFunctionType
    n_chunks = NT // CHUNK              # 8
    ipc = CHUNK // FD                   # iters per chunk = 4
    n_total = NT // FD                  # global iterations = 32

    with tile.TileContext(nc) as tc, ExitStack() as ctx:
        wpool = ctx.enter_context(tc.tile_pool(name="wpool", bufs=1))
        iopool = ctx.enter_context(tc.tile_pool(name="iopool", bufs=2))
        mid = ctx.enter_context(tc.tile_pool(name="mid", bufs=3))
        pp = ctx.enter_context(tc.tile_pool(name="pp", bufs=1, space="PSUM"))

        # --- warm-up scratch (no DMA deps; PE can start immediately) ---
        wu_w = wpool.tile([P, P], BF, tag="wu_w", name="wu_w")
        nc.gpsimd.memset(wu_w[:], 0.0)
        wu_in = wpool.tile([P, FD], BF, tag="wu_in", name="wu_in")
        nc.gpsimd.memset(wu_in[:], 0.0)

        # --- ACT spline tables FIRST on the scalar queue: relu(0) gates the
        # h1 PSUM rotation, so nothing may sit ahead of it on ACT ---
        dum_out = wpool.tile([P, 1], BF, tag="dum_out", name="dum_out")
        nc.scalar.activation(dum_out[:], wu_w[:, 0:1], AF.Relu)
        nc.scalar.activation(dum_out[:], wu_w[:, 0:1], AF.Sigmoid)

        # --- weights / biases resident in SBUF (packed, one DMA each);
        # h1-path weights ride the sync ring ahead of the k slices ---
        wh1_t = wpool.tile([P, NDC, 2 * P], BF, tag="wh1", name="wh1")
        nc.sync.dma_start(wh1_t[:], wh1_d.rearrange("c p e -> p c e"))
        wbn8_t = wpool.tile([P, NDC, P], F8, tag="wbn8", name="wbn8")
        nc.sync.dma_start(wbn8_t[:], wbn8_d[:])
        w2_t = wpool.tile([P, DIM], BF, tag="w2", name="w2")
        nc.scalar.dma_start(w2_t[:], w2_d[:])
        bias_t = wpool.tile([P, 3], F32, tag="bias", name="bias")
        nc.scalar.dma_start(bias_t[:], bias_d[:])
        wvo_t = wpool.tile([P, NDC, 2 * DIM], BF, tag="wvo", name="wvo")
        nc.gpsimd.dma_start(wvo_t[:], wvo_d.rearrange("c p e -> p c e"))

        def a_w(c):
            return wh1_t[:, c, 0:P]

        def w1_w(c):
            return wh1_t[:, c, P:2 * P]

        def wkv_w(c, e):
            return wvo_t[:, c, e * P:(e + 1) * P]

        def wo_w(c, e):
            return wvo_t[:, c, DIM + e * P:DIM + (e + 1) * P]

        # --- HAM warm-up: one accumulation group of dummy matmuls (no
        # per-MM semaphores, back-to-back on the PE) ---
        wu_ps = pp.tile([P, NDC, FD], F32, tag="xo", bufs=1, name="wu_ps")
        for i in range(WARMUP_MMS):
            nc.tensor.matmul(wu_ps[:, 0, :], wu_w[:], wu_in[:],
                             start=(i == 0), stop=(i == WARMUP_MMS - 1))

        io = {}        # chunk -> dict of io tiles
        st = {}        # global iter -> dict of stage tiles

        def load_chunk(ci):
            csl = bass.ts(ci, CHUNK)
            kt = iopool.tile([P, NDC, CHUNK], BF, tag="kt", bufs=3, name="kt")
            qt = iopool.tile([P, NDC, CHUNK], F8, tag="qt", bufs=3, name="qt")
            post = iopool.tile([P, NDC, CHUNK], BF, tag="post", bufs=3, name="post")
            if ci == 0:
                # first 512-token slice lands fastest so s1(0) starts early;
                # scalar/ACT queue is idle at the head, so it carries pos;
                # steady state keeps it DMA-free
                for sl in (slice(0, FD), slice(FD, CHUNK)):
                    nc.sync.dma_start(kt[:, :, sl], k_r[:, :, sl])
                    nc.scalar.dma_start(post[:, :, sl], pos_r[:, :, sl])
                    nc.gpsimd.dma_start(qt[:, :, sl], q8_r[:, :, sl])
            elif ci == 1:
                # head: sync ring already carries wh1+wbn8+k0+k1; pos1 would
                # land last (~20us) and stall tile 4 — scalar ring is free
                nc.sync.dma_start(kt[:], k_r[:, :, csl])
                nc.scalar.dma_start(post[:], pos_r[:, :, csl])
                nc.gpsimd.dma_start(qt[:], q8_r[:, :, csl])
            else:
                nc.sync.dma_start(kt[:], k_r[:, :, csl])
                nc.sync.dma_start(post[:], pos_r[:, :, csl])
                nc.gpsimd.dma_start(qt[:], q8_r[:, :, csl])
            outt = iopool.tile([P, NDC, CHUNK], BF, tag="outt", bufs=2, name="outt")
            io[ci] = {"kt": kt, "qt": qt, "post": post, "outt": outt}

        def s1(j):
            ci, it = divmod(j, ipc)
            t = io[ci]
            tsl = bass.ts(it, FD)
            h1w = pp.tile([P, FD], F32, tag="h1", bufs=2, name="h1w")
            nc.tensor.matmul(h1w[:], a_w(0), t["kt"][:, 0, tsl],
                             start=True, stop=False)
            nc.tensor.matmul(h1w[:], a_w(1), t["kt"][:, 1, tsl],
                             start=False, stop=False)
            # q term: one fp8 DoubleRow matmul contracts both 128-chunks
            nc.tensor.matmul(h1w[:], wbn8_t[:], t["qt"][:, :, tsl],
                             start=False, stop=False,
                             perf_mode=mybir.MatmulPerfMode.DoubleRow)
            nc.tensor.matmul(h1w[:], w1_w(0), t["post"][:, 0, tsl],
                             start=False, stop=False)
            nc.tensor.matmul(h1w[:], w1_w(1), t["post"][:, 1, tsl],
                             start=False, stop=True)
            vw = pp.tile([P, NDC, FD], F32, tag="v", bufs=1, name="vw")
            for e in range(NDC):
                nc.tensor.matmul(vw[:, e, :], wkv_w(0, e), t["kt"][:, 0, tsl],
                                 start=True, stop=False)
                nc.tensor.matmul(vw[:, e, :], wkv_w(1, e), t["kt"][:, 1, tsl],
                                 start=False, stop=True)
            vp_t = mid.tile([P, NDC, FD], BF, tag="vp", bufs=4, name="vp_t")
            nc.vector.tensor_add(vp_t[:], vw[:], t["post"][:, :, tsl])
            st[j] = {"h1": h1w, "vp": vp_t}

        def s2(j):
            # relu -> h2 matmuls -> sigmoid (merged MLP stage)
            s = st[j]
            h1r = mid.tile([P, FD], BF, tag="h1r", bufs=2, name="h1r")
            nc.scalar.activation(h1r[:], s["h1"][:], AF.Relu, bias=bias_t[:, 0:1])
            h2w = pp.tile([P, NDC, FD], F32, tag="h2", bufs=1, name="h2w")
            for e in range(NDC):
                esl = bass.ts(e, P)
                nc.tensor.matmul(h2w[:, e, :], w2_t[:, esl], h1r[:],
                                 start=True, stop=True)
            at_t = mid.tile([P, NDC, FD], BF, tag="at", bufs=2, name="at_t")
            for e in range(NDC):
                nc.scalar.activation(at_t[:, e, :], h2w[:, e, :], AF.Sigmoid,
                                     bias=bias_t[:, 1 + e:2 + e])
            s["at"] = at_t

        def s4(j):
            s = st[j]
            g_t = mid.tile([P, NDC, FD], BF, tag="g", bufs=2, name="g_t")
            # drain tiles: DVE is ~2.3x faster per element than gpsimd and
            # idle there — shortens the end-of-pipeline latency chain
            e0_eng = nc.vector if j >= n_total - 3 else nc.gpsimd
            e0_eng.tensor_mul(g_t[:, 0, :], s["vp"][:, 0, :], s["at"][:, 0, :])
            nc.vector.tensor_mul(g_t[:, 1, :], s["vp"][:, 1, :], s["at"][:, 1, :])
            s["g"] = g_t

        def s5(j):
            ci, it = divmod(j, ipc)
            t = io[ci]
            tsl = bass.ts(it, FD)
            s = st[j]
            xo = pp.tile([P, NDC, FD], F32, tag="xo", bufs=1, name="xo")
            for e in range(NDC):
                nc.tensor.matmul(xo[:, e, :], wo_w(0, e), s["g"][:, 0, :],
                                 start=True, stop=False)
                nc.tensor.matmul(xo[:, e, :], wo_w(1, e), s["g"][:, 1, :],
                                 start=False, stop=True)
            nc.vector.tensor_copy(t["outt"][:, 0, tsl], xo[:, 0, :])
            nc.scalar.copy(t["outt"][:, 1, tsl], xo[:, 1, :])
            del st[j]
            # out DMA: per 1024 tokens; last chunk per 512-half so the final
            # transfer starts right after its own copy, shrinking the tail
            base = ci * CHUNK
            if ci == n_chunks - 1:
                gsl = slice(base + it * FD, base + (it + 1) * FD)
                nc.sync.dma_start(out_r[:, 0, gsl], t["outt"][:, 0, tsl])
                nc.sync.dma_start(out_r[:, 1, gsl], t["outt"][:, 1, tsl])
            elif it % 2 == 1:
                sl2 = slice((it - 1) * FD, (it + 1) * FD)
                nc.sync.dma_start(out_r[:, :, base + (it - 1) * FD:base + (it + 1) * FD],
                                  t["outt"][:, :, sl2])

        for t in range(n_total + 2):
            if t - 2 >= 0 and t - 2 < n_total:
                s4(t - 2)
            if t < n_total:
                if t % ipc == 0:
                    ci = t // ipc
                    if ci == 0:
                        load_chunk(0)
                        load_chunk(1)
                    elif ci + 1 < n_chunks:
                        load_chunk(ci + 1)
                s1(t)
            if t - 1 >= 0 and t - 1 < n_total:
                s2(t - 1)
            if t - 2 >= 0 and t - 2 < n_total:
                s5(t - 2)

    nc.compile()
    return nc


def _get_nc():
    global _CACHED_NC
    if _CACHED_NC is None:
        _CACHED_NC = _build_nc()
    return _CACHED_NC


def _prep_in_maps(q, k, pos, Wq, Wk, Wv, W1, b1, W2, b2, Wo, bo):
    q = np.asarray(q, dtype=np.float32)
    k = np.asarray(k, dtype=np.float32)
    pos = np.asarray(pos, dtype=np.float32)
    Wq32 = np.asarray(Wq, np.float32)
    Wk32 = np.asarray(Wk, np.float32)
    W132 = np.asarray(W1, np.float32)

    A = (Wk32 @ W132).astype(BF16)                    # (256, 128)
    wh1 = np.concatenate([A, W132.astype(BF16)], axis=1)         # (256, 256)
    Bn = -(Wq32 @ W132)                               # (256, 128) fp32
    wbn8 = np.ascontiguousarray(
        Bn.reshape(NDC, P, P).transpose(1, 0, 2).astype(FP8))    # (128, 2, 128)
    Wkv = (Wk32 @ np.asarray(Wv, np.float32)).astype(BF16)   # (256, 256)
    Wob = np.asarray(Wo, np.float32).astype(BF16)
    wvo = np.concatenate([Wkv, Wob], axis=1)          # (256, 512)
    bias = np.stack([np.asarray(b1, np.float32),
                     np.asarray(b2, np.float32)[:P],
                     np.asarray(b2, np.float32)[P:]], axis=1)  # (128, 3)

    weights = {
        "wh1": np.ascontiguousarray(wh1).reshape(NDC, P, 2 * P),
        "wbn8": wbn8,
        "wvo": np.ascontiguousarray(wvo).reshape(NDC, P, 2 * DIM),
        "w2": np.ascontiguousarray(np.asarray(W2, np.float32).astype(BF16)).reshape(P, DIM),
        "bias": np.ascontiguousarray(bias.astype(np.float32)),
    }

    nhalf = N // 2
    in_maps = []
    for c in range(NCORES):
        b = c // 2
        n0 = (c % 2) * nhalf
        qs = q[b, :, n0:n0 + nhalf, :].reshape(DIM, NT)
        ks = k[b, :, n0:n0 + nhalf, :].reshape(DIM, NT)
        ps = np.ascontiguousarray(
            pos[b, n0:n0 + nhalf].reshape(NT, DIM).T
        )
        m = dict(weights)
        m["q8s"] = qs.astype(FP8).reshape(NDC, P, NT)
        m["ks"] = ks.astype(BF16).reshape(NDC, P, NT)
        m["poss"] = ps.astype(BF16).reshape(NDC, P, NT)
        in_maps.append(m)
    return in_maps


def _run(in_maps, trace=False, **kwargs):
    nc = _get_nc()
    return bass_utils.run_bass_kernel_spmd(
        nc, in_maps, core_ids=list(range(NCORES)), trace=trace, **kwargs
    )


def _assemble(results, bo, mask):
    bo = np.asarray(bo, np.float32)
    out = np.empty((B, DIM, N, M), dtype=np.float32)
    nhalf = N // 2
    for c in range(NCORES):
        b = c // 2
        n0 = (c % 2) * nhalf
        r = results[c]["out"].reshape(DIM, nhalf, M).astype(np.float32)
        r += bo[:, None, None]
        out[b, :, n0:n0 + nhalf, :] = r
    mask = np.asarray(mask)
    if not np.all(mask != 0):
        # masked positions: sigmoid(-1e9)=0 -> x=0 -> out = bo
        zb, zn, zm = np.nonzero(mask[..., 0] == 0)
        out[zb, :, zn, zm] = bo[None, :]
    return out


def kernel(q, k, pos, mask, Wq, Wk, Wv, W1, b1, W2, b2, Wo, bo):
    in_maps = _prep_in_maps(q, k, pos, Wq, Wk, Wv, W1, b1, W2, b2, Wo, bo)
    res = _run(in_maps)
    return _assemble(res.results, bo, mask)


def install_profile_hook():
    """Register the axon NTFF profiling hook (antenv.axon_hooks shim) so
    run_bass_kernel_spmd(trace=True) yields exec_time_ns + perfetto trace."""
    import types

    try:
        import antenv.axon_hooks  # noqa: F401
        return True
    except ImportError:
        pass
    try:
        from trn_agent_boot.trn_boot import _ntff_profile_via_ctypes
    except ImportError:
        return False
    hook = _ntff_profile_via_ctypes("/opt/axon/libaxon_pjrt.so")
    if hook is None:
        return False
    mod = types.ModuleType("antenv.axon_hooks")
    mod.get_axon_ntff_profile_hook = lambda: hook
    mod.set_axon_ntff_profile_hook = lambda h: None
    import antenv

    sys.modules["antenv.axon_hooks"] = mod
    antenv.axon_hooks = mod
    # artifact upload has no share reachable from this container
    bass_utils.upload_artifacts = lambda tmpdir: tmpdir
    return True
